# revision 4
# baseline (speedup 1.0000x reference)
"""Tensor x data parallel SwiGLU MLP (LLaMA-style) on 8 Trainium2 NeuronCores.

Problem: y = (silu(x @ Wg^T) * (x @ Wu^T)) @ Wd^T
  x [2, 2048, 4096] f32, Wg/Wu [11008, 4096] f32, Wd [4096, 11008] f32.

Sharding: 2-way tensor-parallel over d_ff (halves of 5504 = 43*128, so NO
padding: 11008 = 86*128 splits exactly in two) x 4-way data-parallel over
tokens (1024 per core). Core c = (d, h) with d = c//2 (token group) and
h = c%2 (d_ff half). Each core computes a full-d_model partial y for its
1024 tokens; the host sums the two halves per token group.

This removes the baseline's d_ff padding (1376 -> 1408 per core = +2.3%
wasted matmuls): per-core matmul count drops 8448 -> 8256 of N=512.

Compute is bf16 on the TensorEngine with f32 PSUM accumulation. Weights
stream once per core (135 MB). Weight loads ride the sync HWDGE queue,
x loads the gpsimd SWDGE queue, outputs the scalar HWDGE queue. A short
block of dummy warmup matmuls trips the PE HAM clock-gate to 2.4 GHz
while the first DMAs land.

kernel(**inputs) -> np.ndarray [2, 2048, 4096] f32.
Set env MLP_KERNEL_TRACE=1 to capture a neuron-profile; the measured
exec_time_ns is then stored in LAST_EXEC_TIME_NS.
"""

import os
import sys
import types

import numpy as np
import ml_dtypes

import concourse.bacc as bacc
import concourse.mybir as mybir
import concourse.tile as tile
from concourse.bass_utils import run_bass_kernel_spmd

P = 128
D = 4096            # d_model
DFF = 11008
NCORES = 8
F2 = DFF // 2       # 5504 per d_ff half (exactly 43 * 128)
NF = F2 // P        # 43 f-chunks per core
TT = 1024           # tokens per core (4-way DP over 4096 tokens)
KD = D // P         # 32 k-subtiles for gate/up
MD = D // P         # 32 output row chunks

NFREE = 512         # matmul moving-dim / PSUM bank size (f32)
NI = TT // NFREE    # 2 token slices
KCH = 16            # k-subtiles per weight DMA chunk (gate/up)
XCH = 8             # x DMA chunks (per ni slice: 4 chunks of 8 k-subtiles)
WARMUP = 120

BF16 = mybir.dt.bfloat16
F32 = mybir.dt.float32
NPBF16 = ml_dtypes.bfloat16

LAST_EXEC_TIME_NS = None
_CACHED_NC = None


def _build():
    nc = bacc.Bacc("TRN2", target_bir_lowering=False, debug=False)

    xh = nc.dram_tensor("xh", [P, NI, KD, NFREE], BF16, kind="ExternalInput")
    wg = nc.dram_tensor("wg", [NF, P, KD, P], BF16, kind="ExternalInput")
    wu = nc.dram_tensor("wu", [NF, P, KD, P], BF16, kind="ExternalInput")
    wd = nc.dram_tensor("wd", [MD, P, NF, P], BF16, kind="ExternalInput")
    y = nc.dram_tensor("y", [MD, P, TT], F32, kind="ExternalOutput")

    silu = mybir.ActivationFunctionType.Silu

    with tile.TileContext(nc) as tc:
        with (
            tc.tile_pool(name="xp", bufs=1) as xp,
            tc.tile_pool(name="wgp", bufs=2) as wgp,
            tc.tile_pool(name="wup", bufs=2) as wup,
            tc.tile_pool(name="wdp", bufs=2) as wdp,
            tc.tile_pool(name="hp", bufs=1) as hp,
            tc.tile_pool(name="gp", bufs=2) as gp,
            tc.tile_pool(name="op", bufs=4) as op,
            tc.tile_pool(name="ps", bufs=2, space="PSUM") as ps,
        ):
            # Warm the PE HAM clock-gate while the first DMAs are in
            # flight: dummy matmuls on a zeroed scratch tile into a
            # scratch PSUM bank nobody reads (shares the pg tag's banks).
            wsc = gp.tile([P, 2 * P], BF16, name="wsc", tag="wsc", bufs=1)
            nc.vector.memset(wsc[:], 0.0)
            pw = ps.tile([P, P], F32, name="pw", tag="pg")
            for _ in range(WARMUP):
                nc.tensor.matmul(pw[:], wsc[:, :P], wsc[:, P:],
                                 start=True, stop=True)

            # Startup cargo rides the scalar HWDGE queue (idle until
            # phase 2) in exact consumption order: fi=0's first weight
            # chunks, then x ni=0, then x ni=1. One queue keeps the
            # early HBM bandwidth undivided, and HWDGE avoids the
            # ~0.6us/DMA Q7 descriptor-emission serialization of the
            # gpsimd SWDGE path.
            w0 = []
            for pool, wsrc in ((wgp, wg), (wup, wu)):
                t1 = pool.tile([P, KCH, P], BF16, name="w0a", tag="wgt"
                               if wsrc is wg else "wut")
                nc.scalar.dma_start(t1[:], wsrc[0, :, 0:KCH, :])
                t2 = pool.tile([P, KCH, P], BF16, name="w0b", tag="wgt"
                               if wsrc is wg else "wut")
                nc.scalar.dma_start(t2[:], wsrc[0, :, KCH:KD, :])
                w0.append((t1, t2))
            # x follows w0 on the scalar queue, all in consumption order
            # (ni=0 then ni=1). Everything startup-critical rides ONE
            # queue: a second active queue halves the critical stream's
            # HBM share (measured), and the sync queue must stay
            # weights-only (ring-gated weight DMAs head-of-line block
            # anything emitted after them). Few, large chunks: per-queue
            # dma_start issue is serialized at ~1us each.
            xt = xp.tile([P, NI, KD, NFREE], BF16, name="xt", tag="xt")
            for ni in range(NI):
                for ks in (slice(0, 11), slice(11, 22), slice(22, 32)):
                    nc.scalar.dma_start(xt[:, ni, ks, :], xh[:, ni, ks, :])

            ht = hp.tile([P, NF, TT], BF16, name="ht", tag="ht")

            NFH = 22  # wd DMA chunk (k-subtiles); 22 + 21 = 43
            wd_pre = []

            # Phase 1: gate/up projections + silu + mul -> ht
            for fi in range(NF):
                if fi == 0:
                    (wgt, wgt2), (wut, wut2) = w0
                else:
                    wgt = wgp.tile([P, KCH, P], BF16, name="wgt", tag="wgt")
                    nc.sync.dma_start(wgt[:], wg[fi, :, 0:KCH, :])
                    wgt2 = wgp.tile([P, KCH, P], BF16, name="wgt2",
                                    tag="wgt")
                    nc.sync.dma_start(wgt2[:], wg[fi, :, KCH:KD, :])
                    wut = wup.tile([P, KCH, P], BF16, name="wut", tag="wut")
                    nc.sync.dma_start(wut[:], wu[fi, :, 0:KCH, :])
                    wut2 = wup.tile([P, KCH, P], BF16, name="wut2",
                                    tag="wut")
                    nc.sync.dma_start(wut2[:], wu[fi, :, KCH:KD, :])
                for ni in range(NI):
                    pg = ps.tile([P, NFREE], F32, name="pg", tag="pg")
                    for k in range(KD):
                        wt = wgt if k < KCH else wgt2
                        nc.tensor.matmul(pg[:], wt[:, k % KCH, :],
                                         xt[:, ni, k, :],
                                         start=(k == 0), stop=(k == KD - 1))
                    pu = ps.tile([P, NFREE], F32, name="pu", tag="pu")
                    for k in range(KD):
                        wt = wut if k < KCH else wut2
                        nc.tensor.matmul(pu[:], wt[:, k % KCH, :],
                                         xt[:, ni, k, :],
                                         start=(k == 0), stop=(k == KD - 1))
                    gt = gp.tile([P, NFREE], BF16, name="gt", tag="gt")
                    nc.scalar.activation(gt[:], pg[:], silu)
                    sl = slice(ni * NFREE, (ni + 1) * NFREE)
                    nc.vector.tensor_mul(ht[:, fi, sl], pu[:], gt[:])
                if fi in (2, 3):
                    # The wdt/wdt2 ring slots are free at t=0, so without
                    # a gate the first two mi's down-proj weights (5.6 MB,
                    # not needed until phase 2) would stream at startup
                    # and steal HBM bandwidth from x. A 1-element copy
                    # that READS ht[fi] (a true data dependency, so the
                    # scheduler cannot hoist it) forces their DMAs to
                    # start only after this fi's mul, ~70us in.
                    mi0 = fi - 2
                    wdta = wdp.tile([P, NFH, P], BF16, name="wdta",
                                    tag="wdt")
                    nc.vector.tensor_copy(wdta[:, 0, 0:1], ht[:, fi, 0:1])
                    nc.sync.dma_start(wdta[:], wd[mi0, :, 0:NFH, :])
                    wdtb = wdp.tile([P, NF - NFH, P], BF16, name="wdtb",
                                    tag="wdt2")
                    nc.vector.tensor_copy(wdtb[:, 0, 0:1], ht[:, fi, 0:1])
                    nc.sync.dma_start(wdtb[:], wd[mi0, :, NFH:NF, :])
                    wd_pre.append((wdta, wdtb))

            # Phase 2: down projection, contraction over the d_ff half
            for mi in range(MD):
                if mi < len(wd_pre):
                    wdt, wdt2 = wd_pre[mi]
                else:
                    wdt = wdp.tile([P, NFH, P], BF16, name="wdt", tag="wdt")
                    nc.sync.dma_start(wdt[:], wd[mi, :, 0:NFH, :])
                    wdt2 = wdp.tile([P, NF - NFH, P], BF16, name="wdt2",
                                    tag="wdt2")
                    nc.sync.dma_start(wdt2[:], wd[mi, :, NFH:NF, :])
                for ni in range(NI):
                    sl = slice(ni * NFREE, (ni + 1) * NFREE)
                    py = ps.tile([P, NFREE], F32, name="py", tag="py",
                                 bufs=4)
                    for k in range(NF):
                        wt = wdt[:, k, :] if k < NFH else wdt2[:, k - NFH, :]
                        nc.tensor.matmul(py[:], wt, ht[:, k, sl],
                                         start=(k == 0), stop=(k == NF - 1))
                    ot = op.tile([P, NFREE], F32, name="ot", tag="ot")
                    # Evictions alternate ACT/DVE and are emitted at high
                    # scheduler priority so they never queue behind ops
                    # that wait on the PE.
                    with tc.high_priority():
                        if ni % 2 == 0:
                            nc.scalar.copy(ot[:], py[:])
                        else:
                            nc.vector.tensor_copy(ot[:], py[:])
                        # outputs on the scalar-engine HWDGE queue so
                        # they never block weight loads on the sync queue
                        nc.scalar.dma_start(y[mi, :, sl], ot[:])

    nc.compile()
    return nc


def _prep_inputs(x, W_gate, W_up, W_down):
    xf = np.ascontiguousarray(np.asarray(x, dtype=np.float32)).reshape(4096, D)
    # xh_d[p, ni, k, t] = x[d*1024 + ni*512 + t, k*128 + p]
    xh = np.ascontiguousarray(
        xf.reshape(4, NI, NFREE, KD, P).transpose(0, 4, 1, 3, 2)
    ).astype(NPBF16)

    Wg = np.asarray(W_gate, dtype=np.float32)
    Wu = np.asarray(W_up, dtype=np.float32)
    Wd = np.asarray(W_down, dtype=np.float32)

    halves = []
    for h in range(2):
        fs = h * F2
        # wg[fi, p, k, j] = Wg[fs + fi*128 + j, k*128 + p]
        wgs = np.ascontiguousarray(
            Wg[fs:fs + F2].reshape(NF, P, KD, P).transpose(0, 3, 2, 1)
        ).astype(NPBF16)
        wus = np.ascontiguousarray(
            Wu[fs:fs + F2].reshape(NF, P, KD, P).transpose(0, 3, 2, 1)
        ).astype(NPBF16)
        # wd[mi, p, k, j] = Wd[mi*128 + j, fs + k*128 + p]
        wds = np.ascontiguousarray(
            Wd[:, fs:fs + F2].reshape(MD, P, NF, P).transpose(0, 3, 2, 1)
        ).astype(NPBF16)
        halves.append((wgs, wus, wds))

    in_maps = []
    for c in range(NCORES):
        d, h = c // 2, c % 2
        wgs, wus, wds = halves[h]
        in_maps.append({"xh": xh[d], "wg": wgs, "wu": wus, "wd": wds})
    return in_maps


def _install_ntff_shim():
    """antenv.axon_hooks is missing from some images; register an
    equivalent module so trace=True can capture NTFF profiles."""
    try:
        import antenv.axon_hooks  # noqa: F401
        return True
    except ImportError:
        pass
    try:
        import antenv
        from trn_agent_boot.trn_boot import _ntff_profile_via_ctypes
        hook = _ntff_profile_via_ctypes('/opt/axon/libaxon_pjrt.so')
        mod = types.ModuleType("antenv.axon_hooks")
        mod._hook = hook
        mod.get_axon_ntff_profile_hook = lambda: mod._hook

        def set_axon_ntff_profile_hook(h):
            mod._hook = h

        mod.set_axon_ntff_profile_hook = set_axon_ntff_profile_hook
        sys.modules["antenv.axon_hooks"] = mod
        antenv.axon_hooks = mod
        return True
    except Exception:
        return False


def kernel(x, W_gate, W_up, W_down):
    global LAST_EXEC_TIME_NS, _CACHED_NC
    if _CACHED_NC is None:
        _CACHED_NC = _build()
    nc = _CACHED_NC

    in_maps = _prep_inputs(x, W_gate, W_up, W_down)

    trace = os.environ.get("MLP_KERNEL_TRACE", "0") == "1"
    if trace:
        trace = _install_ntff_shim()
    tmpdir = os.environ.get("MLP_TRACE_DIR") or None
    if tmpdir:
        os.makedirs(tmpdir, exist_ok=True)

    res = run_bass_kernel_spmd(nc, in_maps, list(range(NCORES)), trace=trace,
                               tmpdir=tmpdir)
    LAST_EXEC_TIME_NS = res.exec_time_ns

    # y[mi, p, t] = y_partial[mi*128 + p, tokens d*1024 + t]; sum the two
    # d_ff halves per token group, then lay out token-major.
    outs = []
    for d in range(4):
        acc = res.results[2 * d]["y"].astype(np.float32, copy=True)
        acc += res.results[2 * d + 1]["y"]
        outs.append(acc.transpose(2, 0, 1).reshape(TT, D))
    yout = np.ascontiguousarray(np.concatenate(outs, axis=0))
    return yout.reshape(2, 2048, D)


# revision 6
# speedup vs baseline: 1.0040x; 1.0040x over previous
"""Tensor x data parallel SwiGLU MLP (LLaMA-style) on 8 Trainium2 NeuronCores.

Problem: y = (silu(x @ Wg^T) * (x @ Wu^T)) @ Wd^T
  x [2, 2048, 4096] f32, Wg/Wu [11008, 4096] f32, Wd [4096, 11008] f32.

Sharding: 2-way tensor-parallel over d_ff (halves of 5504 = 43*128, so NO
padding: 11008 = 86*128 splits exactly in two) x 4-way data-parallel over
tokens (1024 per core). Core c = (d, h) with d = c//2 (token group) and
h = c%2 (d_ff half). Each core computes a full-d_model partial y for its
1024 tokens; the host sums the two halves per token group.

This removes the baseline's d_ff padding (1376 -> 1408 per core = +2.3%
wasted matmuls): per-core matmul count drops 8448 -> 8256 of N=512.

Compute is bf16 on the TensorEngine with f32 PSUM accumulation. Weights
stream once per core (135 MB) on the sync HWDGE queue. All startup-
critical cargo (fi=0 weights, then x) rides the scalar HWDGE queue as a
few large transfers in exact consumption order: the early window is HBM-
bandwidth-bound, so a second active queue or many small DMAs (issue is
serialized ~1us each per queue) delays the first-needed bytes. The first
two down-proj weight tiles are gated behind a read of ht so they cannot
steal startup bandwidth. Outputs ride the scalar queue during phase 2.
A short block of dummy warmup matmuls trips the PE HAM clock-gate to
2.4 GHz while the first DMAs land.

kernel(**inputs) -> np.ndarray [2, 2048, 4096] f32.
Set env MLP_KERNEL_TRACE=1 to capture a neuron-profile; the measured
exec_time_ns is then stored in LAST_EXEC_TIME_NS.
"""

import os
import sys
import types

import numpy as np
import ml_dtypes

import concourse.bacc as bacc
import concourse.mybir as mybir
import concourse.tile as tile
from concourse.bass_utils import run_bass_kernel_spmd

P = 128
D = 4096            # d_model
DFF = 11008
NCORES = 8
F2 = DFF // 2       # 5504 per d_ff half (exactly 43 * 128)
NF = F2 // P        # 43 f-chunks per core
TT = 1024           # tokens per core (4-way DP over 4096 tokens)
KD = D // P         # 32 k-subtiles for gate/up
MD = D // P         # 32 output row chunks

NFREE = 512         # matmul moving-dim / PSUM bank size (f32)
NI = TT // NFREE    # 2 token slices
KCH = 16            # k-subtiles per weight DMA chunk (gate/up)
WARMUP = 120

BF16 = mybir.dt.bfloat16
F32 = mybir.dt.float32
NPBF16 = ml_dtypes.bfloat16

LAST_EXEC_TIME_NS = None
_CACHED_NC = None


def _build():
    nc = bacc.Bacc("TRN2", target_bir_lowering=False, debug=False)

    xh = nc.dram_tensor("xh", [P, NI, KD, NFREE], BF16, kind="ExternalInput")
    wg = nc.dram_tensor("wg", [NF, P, KD, P], BF16, kind="ExternalInput")
    wu = nc.dram_tensor("wu", [NF, P, KD, P], BF16, kind="ExternalInput")
    wd = nc.dram_tensor("wd", [MD, P, NF, P], BF16, kind="ExternalInput")
    y = nc.dram_tensor("y", [MD, P, TT], F32, kind="ExternalOutput")

    silu = mybir.ActivationFunctionType.Silu

    with tile.TileContext(nc) as tc:
        with (
            tc.tile_pool(name="xp", bufs=1) as xp,
            tc.tile_pool(name="wgp", bufs=2) as wgp,
            tc.tile_pool(name="wup", bufs=2) as wup,
            tc.tile_pool(name="wdp", bufs=2) as wdp,
            tc.tile_pool(name="hp", bufs=1) as hp,
            tc.tile_pool(name="gp", bufs=2) as gp,
            tc.tile_pool(name="op", bufs=4) as op,
            tc.tile_pool(name="ps", bufs=2, space="PSUM") as ps,
        ):
            # Warm the PE HAM clock-gate while the first DMAs are in
            # flight: dummy matmuls on a zeroed scratch tile into a
            # scratch PSUM bank nobody reads (shares the pg tag's banks).
            wsc = gp.tile([P, 2 * P], BF16, name="wsc", tag="wsc", bufs=1)
            nc.vector.memset(wsc[:], 0.0)
            pw = ps.tile([P, P], F32, name="pw", tag="pg")
            for _ in range(WARMUP):
                nc.tensor.matmul(pw[:], wsc[:, :P], wsc[:, P:],
                                 start=True, stop=True)

            # Startup cargo rides the scalar HWDGE queue (idle until
            # phase 2) in exact consumption order: fi=0's first weight
            # chunks, then x ni=0, then x ni=1. One queue keeps the
            # early HBM bandwidth undivided, and HWDGE avoids the
            # ~0.6us/DMA Q7 descriptor-emission serialization of the
            # gpsimd SWDGE path.
            w0 = []
            for pool, wsrc in ((wgp, wg), (wup, wu)):
                t1 = pool.tile([P, KCH, P], BF16, name="w0a", tag="wgt"
                               if wsrc is wg else "wut")
                nc.scalar.dma_start(t1[:], wsrc[0, :, 0:KCH, :])
                t2 = pool.tile([P, KCH, P], BF16, name="w0b", tag="wgt"
                               if wsrc is wg else "wut")
                nc.scalar.dma_start(t2[:], wsrc[0, :, KCH:KD, :])
                w0.append((t1, t2))
            # x follows w0 on the scalar queue, all in consumption order
            # (ni=0 then ni=1). Everything startup-critical rides ONE
            # queue: a second active queue halves the critical stream's
            # HBM share (measured), and the sync queue must stay
            # weights-only (ring-gated weight DMAs head-of-line block
            # anything emitted after them). Few, large chunks: per-queue
            # dma_start issue is serialized at ~1us each.
            xt = xp.tile([P, NI, KD, NFREE], BF16, name="xt", tag="xt")
            for ni in range(NI):
                for ks in (slice(0, 11), slice(11, 22), slice(22, 32)):
                    nc.scalar.dma_start(xt[:, ni, ks, :], xh[:, ni, ks, :])

            ht = hp.tile([P, NF, TT], BF16, name="ht", tag="ht")

            NFH = 22  # wd DMA chunk (k-subtiles); 22 + 21 = 43
            wd_pre = []

            # Phase 1: gate/up projections + silu + mul -> ht
            for fi in range(NF):
                if fi == 0:
                    (wgt, wgt2), (wut, wut2) = w0
                else:
                    wgt = wgp.tile([P, KCH, P], BF16, name="wgt", tag="wgt")
                    nc.sync.dma_start(wgt[:], wg[fi, :, 0:KCH, :])
                    wgt2 = wgp.tile([P, KCH, P], BF16, name="wgt2",
                                    tag="wgt")
                    nc.sync.dma_start(wgt2[:], wg[fi, :, KCH:KD, :])
                    wut = wup.tile([P, KCH, P], BF16, name="wut", tag="wut")
                    nc.sync.dma_start(wut[:], wu[fi, :, 0:KCH, :])
                    wut2 = wup.tile([P, KCH, P], BF16, name="wut2",
                                    tag="wut")
                    nc.sync.dma_start(wut2[:], wu[fi, :, KCH:KD, :])
                for ni in range(NI):
                    pg = ps.tile([P, NFREE], F32, name="pg", tag="pg")
                    for k in range(KD):
                        wt = wgt if k < KCH else wgt2
                        nc.tensor.matmul(pg[:], wt[:, k % KCH, :],
                                         xt[:, ni, k, :],
                                         start=(k == 0), stop=(k == KD - 1))
                    pu = ps.tile([P, NFREE], F32, name="pu", tag="pu")
                    for k in range(KD):
                        wt = wut if k < KCH else wut2
                        nc.tensor.matmul(pu[:], wt[:, k % KCH, :],
                                         xt[:, ni, k, :],
                                         start=(k == 0), stop=(k == KD - 1))
                    gt = gp.tile([P, NFREE], BF16, name="gt", tag="gt")
                    nc.scalar.activation(gt[:], pg[:], silu)
                    sl = slice(ni * NFREE, (ni + 1) * NFREE)
                    nc.vector.tensor_mul(ht[:, fi, sl], pu[:], gt[:])
                if fi in (2, 3):
                    # The wdt/wdt2 ring slots are free at t=0, so without
                    # a gate the first two mi's down-proj weights (5.6 MB,
                    # not needed until phase 2) would stream at startup
                    # and steal HBM bandwidth from x. A 1-element copy
                    # that READS ht[fi] (a true data dependency, so the
                    # scheduler cannot hoist it) forces their DMAs to
                    # start only after this fi's mul, ~70us in.
                    mi0 = fi - 2
                    wdta = wdp.tile([P, NFH, P], BF16, name="wdta",
                                    tag="wdt")
                    nc.vector.tensor_copy(wdta[:, 0, 0:1], ht[:, fi, 0:1])
                    nc.sync.dma_start(wdta[:], wd[mi0, :, 0:NFH, :])
                    wdtb = wdp.tile([P, NF - NFH, P], BF16, name="wdtb",
                                    tag="wdt2")
                    nc.vector.tensor_copy(wdtb[:, 0, 0:1], ht[:, fi, 0:1])
                    nc.sync.dma_start(wdtb[:], wd[mi0, :, NFH:NF, :])
                    wd_pre.append((wdta, wdtb))

            # Phase 2: down projection, contraction over the d_ff half
            for mi in range(MD):
                if mi < len(wd_pre):
                    wdt, wdt2 = wd_pre[mi]
                else:
                    wdt = wdp.tile([P, NFH, P], BF16, name="wdt", tag="wdt")
                    nc.sync.dma_start(wdt[:], wd[mi, :, 0:NFH, :])
                    wdt2 = wdp.tile([P, NF - NFH, P], BF16, name="wdt2",
                                    tag="wdt2")
                    nc.sync.dma_start(wdt2[:], wd[mi, :, NFH:NF, :])
                for ni in range(NI):
                    sl = slice(ni * NFREE, (ni + 1) * NFREE)
                    py = ps.tile([P, NFREE], F32, name="py", tag="py",
                                 bufs=4)
                    for k in range(NF):
                        wt = wdt[:, k, :] if k < NFH else wdt2[:, k - NFH, :]
                        nc.tensor.matmul(py[:], wt, ht[:, k, sl],
                                         start=(k == 0), stop=(k == NF - 1))
                    ot = op.tile([P, NFREE], F32, name="ot", tag="ot")
                    # Evictions alternate ACT/DVE and are emitted at high
                    # scheduler priority so they never queue behind ops
                    # that wait on the PE.
                    with tc.high_priority():
                        if ni % 2 == 0:
                            nc.scalar.copy(ot[:], py[:])
                        else:
                            nc.vector.tensor_copy(ot[:], py[:])
                        # outputs on the scalar-engine HWDGE queue so
                        # they never block weight loads on the sync queue
                        nc.scalar.dma_start(y[mi, :, sl], ot[:])

    nc.compile()
    return nc


def _prep_inputs(x, W_gate, W_up, W_down):
    xf = np.ascontiguousarray(np.asarray(x, dtype=np.float32)).reshape(4096, D)
    # xh_d[p, ni, k, t] = x[d*1024 + ni*512 + t, k*128 + p]
    xh = np.ascontiguousarray(
        xf.reshape(4, NI, NFREE, KD, P).transpose(0, 4, 1, 3, 2)
    ).astype(NPBF16)

    Wg = np.asarray(W_gate, dtype=np.float32)
    Wu = np.asarray(W_up, dtype=np.float32)
    Wd = np.asarray(W_down, dtype=np.float32)

    halves = []
    for h in range(2):
        fs = h * F2
        # wg[fi, p, k, j] = Wg[fs + fi*128 + j, k*128 + p]
        wgs = np.ascontiguousarray(
            Wg[fs:fs + F2].reshape(NF, P, KD, P).transpose(0, 3, 2, 1)
        ).astype(NPBF16)
        wus = np.ascontiguousarray(
            Wu[fs:fs + F2].reshape(NF, P, KD, P).transpose(0, 3, 2, 1)
        ).astype(NPBF16)
        # wd[mi, p, k, j] = Wd[mi*128 + j, fs + k*128 + p]
        wds = np.ascontiguousarray(
            Wd[:, fs:fs + F2].reshape(MD, P, NF, P).transpose(0, 3, 2, 1)
        ).astype(NPBF16)
        halves.append((wgs, wus, wds))

    in_maps = []
    for c in range(NCORES):
        d, h = c // 2, c % 2
        wgs, wus, wds = halves[h]
        in_maps.append({"xh": xh[d], "wg": wgs, "wu": wus, "wd": wds})
    return in_maps


def _install_ntff_shim():
    """antenv.axon_hooks is missing from some images; register an
    equivalent module so trace=True can capture NTFF profiles."""
    try:
        import antenv.axon_hooks  # noqa: F401
        return True
    except ImportError:
        pass
    try:
        import antenv
        from trn_agent_boot.trn_boot import _ntff_profile_via_ctypes
        hook = _ntff_profile_via_ctypes('/opt/axon/libaxon_pjrt.so')
        mod = types.ModuleType("antenv.axon_hooks")
        mod._hook = hook
        mod.get_axon_ntff_profile_hook = lambda: mod._hook

        def set_axon_ntff_profile_hook(h):
            mod._hook = h

        mod.set_axon_ntff_profile_hook = set_axon_ntff_profile_hook
        sys.modules["antenv.axon_hooks"] = mod
        antenv.axon_hooks = mod
        return True
    except Exception:
        return False


def kernel(x, W_gate, W_up, W_down):
    global LAST_EXEC_TIME_NS, _CACHED_NC
    if _CACHED_NC is None:
        _CACHED_NC = _build()
    nc = _CACHED_NC

    in_maps = _prep_inputs(x, W_gate, W_up, W_down)

    trace = os.environ.get("MLP_KERNEL_TRACE", "0") == "1"
    if trace:
        trace = _install_ntff_shim()
    tmpdir = os.environ.get("MLP_TRACE_DIR") or None
    if tmpdir:
        os.makedirs(tmpdir, exist_ok=True)

    res = run_bass_kernel_spmd(nc, in_maps, list(range(NCORES)), trace=trace,
                               tmpdir=tmpdir)
    LAST_EXEC_TIME_NS = res.exec_time_ns

    # y[mi, p, t] = y_partial[mi*128 + p, tokens d*1024 + t]; sum the two
    # d_ff halves per token group, then lay out token-major.
    outs = []
    for d in range(4):
        acc = res.results[2 * d]["y"].astype(np.float32, copy=True)
        acc += res.results[2 * d + 1]["y"]
        outs.append(acc.transpose(2, 0, 1).reshape(TT, D))
    yout = np.ascontiguousarray(np.concatenate(outs, axis=0))
    return yout.reshape(2, 2048, D)
